# revision 3
# baseline (speedup 1.0000x reference)
"""AlphaNet forward pass on 8 Trainium2 NeuronCores (data-parallel over batch).

v5 pipeline per core (512 samples, two halves of 256):
  DVE/GpSimd: rolling-window stats with the mod-15 pair trick (pairs
        {i,(i+d) mod 15}), bf16 pre-adds halving every reduce's input.
  ACT : sqrt helpers + conv bias+relu epilogues (phase-B epis all-ACT,
        with block-2/3 helpers spliced between them at safe positions).
  PE  : conv + fc1/fc2/fc3 in bf16, N=256 per half; fc1 for half 0 runs
        while DVE computes the stats of blocks 2/3 (the main overlap win).
  DMA : XBAR transposes (1 per block) on the scalar queue; A/fc1 weights
        streamed twice (once per half) in big chunks on the sync queue.
All per-row constant factors (BatchNorm affine, 1/9, 0.9, 0.3, ret's -1, ...)
are folded into the host-built conv matrix A and per-row bias.
"""
import sys
for _p in ("/opt/trn_rl_repo", "/root/.axon_site/_ro/trn_rl_repo"):
    if _p not in sys.path:
        sys.path.append(_p)

from contextlib import ExitStack

import numpy as np
import ml_dtypes

import concourse.bass as bass
import concourse.tile as tile
from concourse import bacc, mybir
from concourse.bass_utils import run_bass_kernel_spmd

bf16 = ml_dtypes.bfloat16
dt = mybir.dt

# ---- problem constants (hardcoded; must match the AlphaNet reference) ----
NFULL = 4096
NCORES = 8
NSH = NFULL // NCORES        # 512 samples per core
F, W, S = 15, 120, 10
NW = W // S                  # 12
HP = 270                     # stat rows
NROW_PAD, WPAD = 272, 16
GROWS = NROW_PAD * WPAD      # 4352 = 34*128
NGT = GROWS // 128           # 34 transposed-feature tiles
K1 = 43200
K1PAD = 43264                # 338*128
NT = K1PAD // 128            # 338
BN_EPS = 1e-5
NB = NSH // 128              # 4 sample blocks per core
NH = NSH // 2                # 256 samples per half

ACHUNK = 8                   # A tiles per DMA
WCHUNK = 4                   # w1 tiles per DMA
USPLIT = 14                  # f-tiles >= USPLIT hold no corr rows
USPLIT2 = 26                 # f-tiles 14..25 hold ONLY cov rows (ready first)
TSPLIT = 140                 # conv tiles >= TSPLIT read f-tiles >= USPLIT


# ------------------------- host-side preparation -------------------------

def _ref_perm():
    """Map device feat row (mod-15 pair layout) -> reference HP row."""
    II, JJ = np.triu_indices(F, k=1)
    p2r = {(int(i), int(j)): p for p, (i, j) in enumerate(zip(II, JJ))}
    rom = np.zeros(HP, dtype=np.int64)
    for d in range(1, 8):
        for i in range(15):
            j = (i + d) % 15
            a, b = min(i, j), max(i, j)
            r = 15 * (d - 1) + i
            rom[r] = p2r[(a, b)]              # corr rows 0..104
            rom[105 + r] = 105 + p2r[(a, b)]  # cov rows 105..209
    for i in range(60):
        rom[210 + i] = 210 + i                # std/zs/ret/dl unchanged
    return rom


def _row_alpha_beta():
    # mine_row / ref_row scale: corr 1/0.9, cov 9, std 3, zs 10/3, ret 1(+1), dl 1
    alpha = np.zeros(HP)
    beta = np.zeros(HP)
    alpha[0:105] = 1.0 / 0.9
    alpha[105:210] = 9.0
    alpha[210:225] = 3.0
    alpha[225:240] = 10.0 / 3.0
    alpha[240:255] = 1.0
    beta[240:255] = 1.0
    alpha[255:270] = 1.0
    return alpha, beta


def _build_device_inputs(inp):
    gamma = float(inp['bn_gamma'][0]); betab = float(inp['bn_beta'][0])
    mu = float(inp['bn_mean'][0]); var = float(inp['bn_var'][0])
    a = gamma / np.sqrt(var + BN_EPS)
    b = betab - mu * a
    conv_w = np.asarray(inp['conv_w'], np.float64).reshape(16, 3)
    conv_b = np.asarray(inp['conv_b'], np.float64)

    alpha, beta = _row_alpha_beta()
    sA = a / alpha
    sB = b - a * beta / alpha

    ybias = np.zeros(K1PAD, np.float64)
    wsum = conv_w.sum(axis=1)
    for mh in range(HP):
        ybias[mh * 160:(mh + 1) * 160] = np.repeat(conv_b + wsum * sB[mh], 10)
    ybias2d = ybias.reshape(NT, 128).T.astype(np.float32).copy()

    rom = _ref_perm()
    m = np.arange(K1)
    mh = m // 160; o = (m % 160) // 10; wp = m % 10
    kref = o * 2700 + rom[mh] * 10 + wp
    fc1_w = np.asarray(inp['fc1_w'], np.float32)
    fc1t = np.zeros((K1PAD, 512), np.float32)
    fc1t[:K1, :] = fc1_w[:, kref].T
    # relayout: fc1r[p, t*512 + j] = fc1t[128 t + p, j]  (chunked t-contiguous)
    fc1r = np.ascontiguousarray(
        fc1t.reshape(NT, 128, 512).transpose(1, 0, 2).reshape(128, NT * 512)
    ).astype(bf16)

    # conv pieces: one full-height (K=128) matmul per tile against f-tile u
    piece_refs = []
    A_packed = np.zeros((NT, 128, 128), np.float32)
    for t in range(NT):
        r0, r1 = 128 * t, 128 * t + 127
        h0 = r0 // 160
        hl = min(r1 // 160, HP - 1)
        hs = list(range(h0, hl + 1))
        u = h0 // 8                       # f tile holds h in [8u, 8u+8)
        assert all(8 * u <= h < 8 * u + 8 for h in hs), (t, hs, u)
        for col in range(128):
            mm = 128 * t + col
            if mm >= K1:
                continue
            mhh = mm // 160
            if mhh not in hs:
                continue
            oo = (mm % 160) // 10
            wpp = mm % 10
            for k in range(3):
                A_packed[t, (mhh - 8 * u) * 16 + wpp + k, col] = \
                    conv_w[oo, k] * sA[mhh]
        piece_refs.append(u)
    # relayout: A2[p, t*128 + m] = A_packed[t, p, m]
    A2 = np.ascontiguousarray(
        A_packed.transpose(1, 0, 2).reshape(128, NT * 128)).astype(bf16)

    fc1b2d = np.asarray(inp['fc1_b'], np.float32).reshape(4, 128).T.copy()
    fc2t = np.ascontiguousarray(
        np.asarray(inp['fc2_w'], np.float32).T).astype(bf16)
    fc2b = np.asarray(inp['fc2_b'], np.float32).reshape(128, 1).copy()
    fc3t = np.ascontiguousarray(
        np.asarray(inp['fc3_w'], np.float32).reshape(1, 128).T).astype(bf16)
    fc3b = np.asarray(inp['fc3_b'], np.float32).reshape(1, 1).copy()
    wdl = np.tile((np.arange(1, 11, dtype=np.float32) / 55.0)[None, :],
                  (128, 1)).astype(bf16)

    return dict(A2=A2, fc1r=fc1r, piece_refs=piece_refs,
                ybias2d=ybias2d, fc1b2d=fc1b2d,
                fc2t=fc2t, fc2b=fc2b, fc3t=fc3t, fc3b=fc3b, wdl=wdl)


# ------------------------- device kernel builder -------------------------

def build_nc(piece_refs, stage=3):
    nc = bacc.Bacc("TRN2", target_bir_lowering=False, debug=False,
                   num_devices=NCORES)
    f32, b16 = dt.float32, dt.bfloat16
    data_e = nc.declare_dram_parameter("data", [NSH, F * W], b16, isOutput=False)
    A_e = nc.declare_dram_parameter("A2", [128, NT * 128], b16, isOutput=False)
    fc1r_e = nc.declare_dram_parameter("fc1r", [128, NT * 512], b16, isOutput=False)
    yb_e = nc.declare_dram_parameter("ybias2d", [128, NT], f32, isOutput=False)
    fc1b_e = nc.declare_dram_parameter("fc1b2d", [128, 4], f32, isOutput=False)
    fc2t_e = nc.declare_dram_parameter("fc2t", [512, 128], b16, isOutput=False)
    fc2b_e = nc.declare_dram_parameter("fc2b", [128, 1], f32, isOutput=False)
    fc3t_e = nc.declare_dram_parameter("fc3t", [128, 1], b16, isOutput=False)
    fc3b_e = nc.declare_dram_parameter("fc3b", [1, 1], f32, isOutput=False)
    wdl_e = nc.declare_dram_parameter("wdl", [128, 10], b16, isOutput=False)
    out_e = nc.declare_dram_parameter("out", [1, NSH], f32, isOutput=True)

    AF = mybir.ActivationFunctionType

    with tile.TileContext(nc) as tc, ExitStack() as ctx:
        consts = ctx.enter_context(tc.tile_pool(name="consts", bufs=1))
        fpool = ctx.enter_context(tc.tile_pool(name="fpool", bufs=1))
        datap = ctx.enter_context(tc.tile_pool(name="datap", bufs=4))
        featp = ctx.enter_context(tc.tile_pool(name="featp", bufs=2))
        nsprp = ctx.enter_context(tc.tile_pool(name="nsprp", bufs=2))
        prodp = ctx.enter_context(tc.tile_pool(name="prodp", bufs=1))
        statp = ctx.enter_context(tc.tile_pool(name="statp", bufs=2))
        xpool = ctx.enter_context(tc.tile_pool(name="xpool", bufs=10))
        w1pool = ctx.enter_context(tc.tile_pool(name="w1pool", bufs=6))
        apool = ctx.enter_context(tc.tile_pool(name="apool", bufs=4))
        x2pool = ctx.enter_context(tc.tile_pool(name="x2pool", bufs=2))
        outp = ctx.enter_context(tc.tile_pool(name="outp", bufs=2))
        ps_fc1 = ctx.enter_context(tc.tile_pool(name="ps_fc1", bufs=1, space="PSUM"))
        ps_conv = ctx.enter_context(tc.tile_pool(name="ps_conv", bufs=4, space="PSUM"))

        # data DMAs first so stats start immediately
        dtiles = []
        for bkl in range(NB):
            d = datap.tile([128, F, NW, S], b16, tag="d", name=f"d{bkl}")
            nc.sync.dma_start(
                d[:], data_e[128 * bkl:128 * (bkl + 1), :]
                .rearrange("p (f nw s) -> p f nw s", f=F, nw=NW))
            dtiles.append(d)

        # constants
        yb_sb = consts.tile([128, NT], f32)
        nc.sync.dma_start(yb_sb[:], yb_e[:])
        fc1b_sb = consts.tile([128, 4], f32)
        nc.sync.dma_start(fc1b_sb[:], fc1b_e[:])
        fc2t_sb = consts.tile([128, 4, 128], b16)
        nc.sync.dma_start(fc2t_sb[:], fc2t_e.rearrange("(kb k) j -> k kb j", k=128))
        fc2b_sb = consts.tile([128, 1], f32)
        nc.sync.dma_start(fc2b_sb[:], fc2b_e[:])
        fc3t_sb = consts.tile([128, 1], b16)
        nc.sync.dma_start(fc3t_sb[:], fc3t_e[:])
        fc3b_sb = consts.tile([1, 1], f32)
        nc.sync.dma_start(fc3b_sb[:], fc3b_e[:])
        wdl_sb = consts.tile([128, 10], b16)
        nc.sync.dma_start(wdl_sb[:], wdl_e[:])

        # persistent bf16 transposed-feature buffer, block-major:
        # f_sb[p, b, u, c] = feat_{block b, sample c}[row-w flat 128u + p]
        f_sb = fpool.tile([128, NB, NGT, 128], b16)

        # -------- per-block stats; ACT helpers optionally deferred --------
        def stats_block(bkl, defer):
            """Emit stats for block bkl. ACT sqrt/copy + the XBAR transpose
            are emitted immediately when defer is None, else appended to
            `defer` as closures (spliced into the ACT queue later)."""
            d = dtiles[bkl]
            feat = featp.tile([128, NROW_PAD, WPAD], b16)
            nc.gpsimd.memset(feat[:, :, NW:WPAD], 0.0)
            nc.gpsimd.memset(feat[:, HP:NROW_PAD, 0:NW], 0.0)

            dh = statp.tile([128, F, NW, 5], b16, tag="dh")
            nc.vector.tensor_add(dh[:], d[:, :, :, 0:5], d[:, :, :, 5:10])
            meansum = statp.tile([128, F, NW], f32, tag="msum")
            nc.vector.tensor_reduce(meansum[:], dh[:], axis=mybir.AxisListType.X,
                                    op=mybir.AluOpType.add)
            mean_b = statp.tile([128, F, NW], b16, tag="meanb")
            nc.vector.tensor_scalar_mul(mean_b[:], meansum[:], 1.0 / S)
            # nspread = mean - d (sign cancels in all downstream products)
            nspread = nsprp.tile([128, F + 7, NW, S], b16)
            nc.vector.tensor_sub(
                nspread[:, 0:F],
                mean_b[:, :, :, None].to_broadcast((128, F, NW, S)), d[:])
            nc.vector.tensor_copy(nspread[:, F:F + 7], nspread[:, 0:7])

            # prodc rows [dd*F + i] = nspread[i] * nspread[i+dd], dd in 0..7
            prodc = prodp.tile([128, 8 * F, NW, S], b16)
            for dd in range(8):
                nc.vector.tensor_mul(prodc[:, dd * F:(dd + 1) * F],
                                     nspread[:, 0:F], nspread[:, dd:dd + F])
            ph = prodp.tile([128, 8 * F, NW, 5], b16, tag="ph")
            nc.vector.tensor_add(ph[:], prodc[:, :, :, 0:5],
                                 prodc[:, :, :, 5:10])
            varsum = statp.tile([128, F, NW], f32, tag="vsum")
            nc.vector.tensor_reduce(varsum[:], ph[:, 0:F],
                                    axis=mybir.AxisListType.X,
                                    op=mybir.AluOpType.add)
            with nc.allow_low_precision(reason="10-elem sums; emu-validated"):
                nc.vector.tensor_reduce(feat[:, 105:210, 0:NW], ph[:, F:8 * F],
                                        axis=mybir.AxisListType.X,
                                        op=mybir.AluOpType.add)

            featf = feat.rearrange("p r w -> p (r w)")
            xeng = nc.scalar if defer is None else nc.sync

            # f-tiles 14..25 hold only cov rows -> transpose as soon as the
            # cov reduce lands (this gates the start of phase B)
            def xbar1a():
                xeng.dma_start_transpose(
                    f_sb[:, bkl, USPLIT:USPLIT2],
                    featf[:, 128 * USPLIT:128 * USPLIT2])
            if defer is None:
                xbar1a()
            else:
                defer.append(xbar1a)

            # std (mine = sqrt(varsum)); rstd = 1/std
            stdf = statp.tile([128, F, NW], f32, tag="stdf")

            def act_helpers():
                nc.scalar.activation(stdf[:], varsum[:], AF.Sqrt,
                                     bias=0.0, scale=1.0)
                nc.scalar.activation(feat[:, 210:225, 0:NW], stdf[:], AF.Copy,
                                     bias=0.0, scale=1.0)
            if defer is None:
                act_helpers()
            else:
                defer.append(act_helpers)
            rstd = statp.tile([128, F + 7, NW], f32, tag="rstd")
            nc.vector.reciprocal(rstd[:, 0:F], stdf[:])
            nc.vector.tensor_copy(rstd[:, F:F + 7], rstd[:, 0:7])
            # zscore (mine = meansum * rstd)
            nc.gpsimd.tensor_mul(feat[:, 225:240, 0:NW], meansum[:], rstd[:, 0:F])
            # return: last/first
            recipf = statp.tile([128, F, NW], f32, tag="recf")
            nc.vector.reciprocal(recipf[:], d[:, :, :, 0])
            nc.gpsimd.tensor_mul(feat[:, 240:255, 0:NW], d[:, :, :, S - 1],
                                 recipf[:])
            # decay-linear
            dlp = statp.tile([128, F, NW, S], b16, tag="dlp")
            nc.gpsimd.tensor_mul(
                dlp[:], d[:], wdl_sb[:, None, None, :].to_broadcast((128, F, NW, S)))
            dlh = statp.tile([128, F, NW, 5], b16, tag="dlh")
            nc.vector.tensor_add(dlh[:], dlp[:, :, :, 0:5], dlp[:, :, :, 5:10])
            with nc.allow_low_precision(reason="10-elem sums; emu-validated"):
                nc.vector.tensor_reduce(feat[:, 255:270, 0:NW], dlh[:],
                                        axis=mybir.AxisListType.X,
                                        op=mybir.AluOpType.add)

            # f-tiles 26..33 (std/zs/ret/dl rows) after the dl reduce
            def xbar1():
                xeng.dma_start_transpose(
                    f_sb[:, bkl, USPLIT2:NGT], featf[:, 128 * USPLIT2:GROWS])
            if defer is None:
                xbar1()
            else:
                defer.append(xbar1)

            # corr rows: cov * rstd_i * rstd_j
            rsp = statp.tile([128, 7, F, NW], f32, tag="rsp")
            for dd in range(1, 8):
                nc.gpsimd.tensor_mul(rsp[:, dd - 1], rstd[:, 0:F],
                                     rstd[:, dd:dd + F])
            nc.gpsimd.tensor_mul(
                feat[:, 0:105, 0:NW],
                feat[:, 105:210, 0:NW],
                rsp.rearrange("p d f w -> p (d f) w"))

            def xbar2():
                xeng.dma_start_transpose(
                    f_sb[:, bkl, 0:USPLIT], featf[:, 0:128 * USPLIT])
            if defer is None:
                xbar2()
            else:
                defer.append(xbar2)

        # blocks 0/1 fully now; blocks 2/3 with ACT helpers + XBARs deferred
        deferred = []
        stats_block(0, None)
        stats_block(1, None)
        stats_block(2, deferred)
        stats_block(3, deferred)
        # deferred = [xb1a_b2, h_b2, xb1_b2, xb2_b2, xb1a_b3, h_b3, xb1_b3,
        # xb2_b3]; splice points are emission indices within the phase-B
        # stream. Helpers run on ACT (tiny); the deferred XBARs issue from
        # the sync queue, which runs ahead of the PE and absorbs waits.
        act_splice = {100: deferred[0], 65: deferred[1], 125: deferred[2],
                      140: deferred[3], 165: deferred[4], 145: deferred[5],
                      185: deferred[6], 215: deferred[7]}

        # ---------------- conv + fc1, one pass per half ----------------
        LEAD = 2

        # tiles >= TSPLIT first: their f-tiles skip the late corr rows
        t_order = list(range(TSPLIT, NT)) + list(range(0, TSPLIT))

        def half_pass(h, epi_engines, splice):
            fc1ps = [ps_fc1.tile([128, NSH], f32, tag=f"jb{jb}",
                                 name=f"fc1ps{jb}_h{h}")
                     for jb in range(4)]
            achunks = {}
            wchunks = {}
            xts = [None] * NT
            for idx in range(NT + LEAD):
                if idx < NT:
                    t = t_order[idx]
                    if idx == NT - TSPLIT:
                        achunks.clear()   # A chunk 17 straddles the reorder
                    ac, ai = t // ACHUNK, t % ACHUNK
                    if ac not in achunks:
                        na = min(ACHUNK, NT - ac * ACHUNK)
                        a1 = apool.tile([128, na, 128], b16, tag="achunk",
                                        name=f"ach{h}_{ac}_{idx}")
                        nc.sync.dma_start(
                            a1[:], A_e[:, 128 * ACHUNK * ac:
                                       128 * (ACHUNK * ac + na)]
                            .rearrange("p (n m) -> p n m", m=128))
                        achunks[ac] = a1
                    wc, wi = t // WCHUNK, t % WCHUNK
                    if wc not in wchunks:
                        nw_ = min(WCHUNK, NT - wc * WCHUNK)
                        w1 = w1pool.tile([128, nw_, 512], b16, tag="wchunk",
                                         name=f"wch{h}_{wc}")
                        nc.sync.dma_start(
                            w1[:], fc1r_e[:, 512 * WCHUNK * wc:
                                          512 * (WCHUNK * wc + nw_)]
                            .rearrange("p (n j) -> p n j", j=512))
                        wchunks[wc] = w1

                    cps = ps_conv.tile([128, NSH], f32, tag="cps",
                                       name=f"cps{h}_{t}")
                    nc.tensor.matmul(cps[:, 0:NH], achunks[ac][:, ai],
                                     f_sb[:, 2 * h:2 * h + 2, piece_refs[t], :],
                                     start=True, stop=True)
                    xt = xpool.tile([128, NH], b16)
                    eng = epi_engines[idx % len(epi_engines)]
                    if splice and idx >= 190 and idx % 3 == 2:
                        eng = nc.vector   # DVE is free once block-3 stats end
                    if eng is nc.scalar:
                        nc.scalar.activation(xt[:], cps[:, 0:NH], AF.Relu,
                                             bias=yb_sb[:, t:t + 1], scale=1.0)
                    else:
                        eng.tensor_scalar(xt[:], cps[:, 0:NH],
                                          yb_sb[:, t:t + 1], 0.0,
                                          op0=mybir.AluOpType.add,
                                          op1=mybir.AluOpType.max)
                    xts[t] = xt
                    if splice and idx in splice:
                        splice[idx]()
                if idx >= LEAD:
                    tt = t_order[idx - LEAD]
                    wc, wi = tt // WCHUNK, tt % WCHUNK
                    for jb in range(4):
                        nc.tensor.matmul(fc1ps[jb][:, 0:NH],
                                         wchunks[wc][:, wi,
                                                     128 * jb:128 * (jb + 1)],
                                         xts[tt][:], start=(idx - LEAD == 0),
                                         stop=(idx - LEAD == NT - 1))
            return fc1ps

        def tail(h, fc1ps):
            c0 = NH * h
            x2 = x2pool.tile([128, 4, NH], b16, tag="x2", name=f"x2_h{h}")
            for jb in range(4):
                nc.scalar.activation(x2[:, jb, :], fc1ps[jb][:, 0:NH], AF.Relu,
                                     bias=fc1b_sb[:, jb:jb + 1], scale=1.0)
            fc2ps = ps_conv.tile([128, NSH], f32, tag="cps", name=f"fc2ps{h}")
            for kb in range(4):
                nc.tensor.matmul(fc2ps[:, 0:NH], fc2t_sb[:, kb, :], x2[:, kb, :],
                                 start=(kb == 0), stop=(kb == 3))
            x3 = x2pool.tile([128, NH], b16, tag="x3", name=f"x3_h{h}")
            nc.scalar.activation(x3[:], fc2ps[:, 0:NH], AF.Sigmoid,
                                 bias=fc2b_sb[:], scale=1.0)
            fc3ps = ps_conv.tile([128, NSH], f32, tag="cps", name=f"fc3ps{h}")
            nc.tensor.matmul(fc3ps[0:1, 0:NH], fc3t_sb[:], x3[:],
                             start=True, stop=True)
            out_sb = outp.tile([1, NH], f32, tag="osb", name=f"osb{h}")
            nc.scalar.activation(out_sb[:], fc3ps[0:1, 0:NH], AF.Identity,
                                 bias=fc3b_sb[:], scale=1.0)
            nc.sync.dma_start(out_e[:, c0:c0 + NH], out_sb[:])

        # phase B: half 0, epis all on ACT (DVE is busy with blocks 2/3)
        fc1ps_h0 = half_pass(0, [nc.scalar], act_splice)
        tail(0, fc1ps_h0)
        # phase C: half 1, epis split DVE/ACT
        fc1ps_h1 = half_pass(1, [nc.vector, nc.scalar], None)
        tail(1, fc1ps_h1)

    nc.compile()
    return nc


# ------------------------------- entry -------------------------------

def _prep_in_maps(inputs):
    dev = _build_device_inputs(inputs)
    data = np.ascontiguousarray(
        np.asarray(inputs['data'], np.float32).reshape(NFULL, F * W)).astype(bf16)
    shared = {k: dev[k] for k in ('A2', 'fc1r', 'ybias2d', 'fc1b2d',
                                  'fc2t', 'fc2b', 'fc3t', 'fc3b', 'wdl')}
    in_maps = []
    for c in range(NCORES):
        m = dict(shared)
        m['data'] = data[NSH * c:NSH * (c + 1)]
        in_maps.append(m)
    return dev, in_maps


def run(inputs, trace=False, tmpdir=None):
    dev, in_maps = _prep_in_maps(inputs)
    nc = build_nc(dev['piece_refs'])
    res = run_bass_kernel_spmd(nc, in_maps, core_ids=list(range(NCORES)),
                               trace=trace, tmpdir=tmpdir)
    out = np.concatenate([np.asarray(r["out"], np.float32).reshape(NSH)
                          for r in res.results])
    return out, res


def kernel(**inputs) -> np.ndarray:
    out, _ = run(inputs, trace=False)
    return out
